# revision 1
# baseline (speedup 1.0000x reference)
"""DiversityAttention on 8 TRN2 NeuronCores (Bass/Tile).

Sharding: data-parallel over batch (B=2) x tensor-parallel over heads
(16 heads -> 4 groups of 4). core = (b, g), b = core // 4, g = core % 4.
Each core computes full attention for its 4 heads over its batch and a
partial out-projection [S, HIDDEN]; the host sums the 4 partials per
batch and adds bo.

Device-side formulation (keys-on-partitions / "S^T" orientation, so no
large transposes are ever needed):
  qT = (Wq/8 @ x^T + bq/8)   [64h, S]   (1/sqrt(dh) folded into Wq on host)
  kT = (Wk   @ x^T + bk)     [64h, S]
  vT = (Wv   @ x^T + bv) then PE-transposed to V [S, 64h] (+ ones col)
  xh = sqrt(gamma) * x^T / max(||x||, eps)  (in-place columns scale of xT)
  per (ktile, qblock): sim_psum[k,q] = xh^T xh ; per head:
     scores_psum[k,q] = kT^T qT ; P = exp(scores - sim) (DVE sub, ACT exp)
  ctx^T[d,q] (+sums row) = sum_k [V|1]^T P  accumulated in PSUM
  ctx normalized by reciprocal(sums) (PE broadcast outer product)
  out[q,o] partial = ctxT^T @ WoT  -> DMA to DRAM

All matmuls run as float32r (full PE rate at N>=256, ~fp32 precision).
"""

import math
import os
import sys

import numpy as np

for _p in ("/opt/trn_rl_repo",):
    if _p not in sys.path and os.path.isdir(_p):
        sys.path.insert(0, _p)

os.environ.setdefault("MYCRO_LOCAL_CACHE", "1")

import concourse.bass as bass
import concourse.tile as tile
from concourse import bacc, mybir
from concourse.bass_utils import run_bass_kernel_spmd
from concourse.masks import make_identity


def _install_ntff_hook():
    """Provide antenv.axon_hooks (NTFF profiling registry) if the image
    lacks it, mirroring trn_agent_boot's ctypes hook. No-op on failure."""
    try:
        import antenv.axon_hooks  # noqa: F401
        return
    except ImportError:
        pass
    try:
        import contextlib
        import ctypes
        import types

        so_path = "/opt/axon/libaxon_pjrt.so"
        if not os.path.exists(so_path):
            return
        lib = ctypes.CDLL(so_path)
        if not hasattr(lib, "axon_start_nrt_profile"):
            return
        lib.axon_start_nrt_profile.argtypes = [
            ctypes.POINTER(ctypes.c_int64), ctypes.c_size_t]
        lib.axon_start_nrt_profile.restype = ctypes.c_int64
        lib.axon_stop_nrt_profile.argtypes = [ctypes.c_char_p]
        lib.axon_stop_nrt_profile.restype = ctypes.c_int64

        @contextlib.contextmanager
        def _hook(output_dir, device_ids):
            import jax
            jax.devices()
            if device_ids:
                ids = (ctypes.c_int64 * len(device_ids))(*device_ids)
                rc = lib.axon_start_nrt_profile(ids, len(device_ids))
            else:
                rc = lib.axon_start_nrt_profile(None, 0)
            if rc != 0:
                raise RuntimeError(f"axon_start_nrt_profile rc={rc}")
            try:
                yield
            finally:
                n = lib.axon_stop_nrt_profile(str(output_dir).encode())
                print(f"ntff profile: {n} file(s) -> {output_dir}",
                      file=sys.stderr)

        mod = types.ModuleType("antenv.axon_hooks")
        _state = {"hook": _hook}
        mod.set_axon_ntff_profile_hook = lambda h: _state.__setitem__("hook", h)
        mod.get_axon_ntff_profile_hook = lambda: _state["hook"]
        sys.modules["antenv.axon_hooks"] = mod
        import antenv
        antenv.axon_hooks = mod
    except Exception:
        pass


_install_ntff_hook()

F32 = mybir.dt.float32
F32R = mybir.dt.float32r
ALU = mybir.AluOpType
ACT_EXP = mybir.ActivationFunctionType.Exp
ACT_COPY = mybir.ActivationFunctionType.Copy

# Problem constants (hardcoded per contract).
HIDDEN = 1024
HEADS = 16
HEAD_DIM = 64
GAMMA = 0.5
B, S = 2, 2048
N_CORES = 8
GROUPS = N_CORES // B  # head groups per batch
HPC = HEADS // GROUPS  # heads per core
LAG = 2  # kt software-pipeline lag between exp and ctx matmul


def _r(ap):
    return ap.bitcast(F32R)


def emit_kernel(tc, aps, *, S_, C_, HPC_, QB):
    """Emit the per-core kernel. aps: dict of dram APs."""
    nc = tc.nc
    CT = C_ // 128          # contraction tiles over hidden
    PAIRS = HPC_ // 2       # head pairs (128-channel chunks)
    NKT = S_ // 128         # key tiles
    NQB = S_ // QB          # query blocks
    PB = min(512, S_)       # projection free-block width
    NPB = S_ // PB
    OB_W = min(512, C_)     # out-projection free-block width
    NOB = C_ // OB_W

    xT_d = aps["xT"]; scale_d = aps["scale"]
    wq_d = aps["wq"]; wk_d = aps["wk"]; wv_d = aps["wv"]; wo_d = aps["wo"]
    bq_d = aps["bq"]; bk_d = aps["bk"]; bv_d = aps["bv"]
    out_d = aps["out"]
    mask_d = aps.get("maskadd")

    from contextlib import ExitStack
    stack = ExitStack()
    consts = stack.enter_context(tc.tile_pool(name="consts", bufs=1))
    xpool = stack.enter_context(tc.tile_pool(name="xpool", bufs=1))
    projpool = stack.enter_context(tc.tile_pool(name="projpool", bufs=1))

    # --- constants ---
    identity = consts.tile([128, 128], F32)
    make_identity(nc, identity)

    wo_sb = consts.tile([128, PAIRS, C_], F32R)

    # x^T loaded in chunks and rounded to fp32r by the scalar engine
    xTr = xpool.tile([128, CT, S_], F32R)

    # projections (fp32r for q/k; plain f32 for v which feeds the transpose)
    qT_sb = projpool.tile([128, PAIRS, S_], F32R)
    kT_sb = projpool.tile([128, PAIRS, S_], F32R)
    v2_sb = projpool.tile([128, HPC_, NKT, HEAD_DIM + 1], F32R)

    with tc.tile_pool(name="xstage", bufs=2) as xstage, \
         tc.tile_pool(name="wstage", bufs=1) as wstage, \
         tc.tile_pool(name="wpool", bufs=1) as wpool, \
         tc.tile_pool(name="vstage", bufs=1) as vstage, \
         tc.tile_pool(name="ph1psum", bufs=2, space="PSUM") as prj_ps, \
         tc.tile_pool(name="tppsum", bufs=4, space="PSUM") as tp_ps:
        # load + round x^T
        for c in range(CT):
            xs = xstage.tile([128, S_], F32, tag="xs")
            nc.sync.dma_start(out=xs, in_=xT_d[c * 128:(c + 1) * 128, :])
            nc.scalar.activation(out=xTr[:, c, :], in_=xs, func=ACT_COPY)
        # load + round weights (DVE)
        wq_sb = wpool.tile([128, CT, D2_of(HPC_)], F32R)
        wk_sb = wpool.tile([128, CT, D2_of(HPC_)], F32R)
        wv_sb = wpool.tile([128, CT, D2_of(HPC_)], F32R)
        for w_sb, w_d in ((wq_sb, wq_d), (wk_sb, wk_d), (wv_sb, wv_d)):
            ws = wstage.tile([128, CT, D2_of(HPC_)], F32, tag="ws")
            nc.sync.dma_start(out=ws, in_=w_d.rearrange("(t p) m -> p t m", p=128))
            nc.vector.tensor_copy(w_sb, ws)
        wos = wstage.tile([128, PAIRS, C_], F32, tag="ws")
        nc.sync.dma_start(out=wos, in_=wo_d.rearrange("(j p) o -> p j o", p=128))
        nc.vector.tensor_copy(wo_sb, wos)
        bq_sb = wpool.tile([128, PAIRS, 1], F32)
        bk_sb = wpool.tile([128, PAIRS, 1], F32)
        bv_sb = wpool.tile([128, PAIRS, 1], F32)
        for b_sb, b_d in ((bq_sb, bq_d), (bk_sb, bk_d), (bv_sb, bv_d)):
            nc.sync.dma_start(
                out=b_sb, in_=b_d.rearrange("(j p) one -> p j one", p=128))

        vT_sb = vstage.tile([128, PAIRS, S_], F32)
        for w_sb, b_sb, dest in (
            (wq_sb, bq_sb, qT_sb),
            (wk_sb, bk_sb, kT_sb),
            (wv_sb, bv_sb, vT_sb),
        ):
            for nb in range(NPB):
                pss = [prj_ps.tile([128, PB], F32, tag=f"prj{j}",
                                   name=f"prj_{dest.tensor.name}_{nb}_{j}")
                       for j in range(PAIRS)]
                for c in range(CT):
                    for j in range(PAIRS):
                        nc.tensor.matmul(
                            pss[j],
                            w_sb[:, c, j * 128:(j + 1) * 128],
                            xTr[:, c, nb * PB:(nb + 1) * PB],
                            start=(c == 0),
                            stop=(c == CT - 1),
                        )
                for j in range(PAIRS):
                    nc.vector.tensor_scalar_add(
                        dest[:, j, nb * PB:(nb + 1) * PB], pss[j], b_sb[:, j, :]
                    )

        # V: PE-transpose vT (f32) -> [keys, d] layout, 2 heads per tile
        for j in range(PAIRS):
            for t in range(NKT):
                tp = tp_ps.tile([128, 128], F32, tag="tp")
                nc.tensor.transpose(tp, vT_sb[:, j, t * 128:(t + 1) * 128], identity)
                nc.scalar.activation(
                    out=v2_sb[:, 2 * j:2 * j + 2, t, 0:HEAD_DIM],
                    in_=tp.rearrange("p (h d) -> p h d", h=2),
                    func=ACT_COPY,
                )
        onescol = wstage.tile([128, HPC_, NKT, 1], F32)
        nc.vector.memset(onescol, 1.0)
        nc.vector.tensor_copy(v2_sb[:, :, :, HEAD_DIM:HEAD_DIM + 1], onescol)

    # xT -> xh in place: multiply columns by sqrt(gamma)/||x_row||
    ctxT2_sb = projpool.tile([128, PAIRS, S_], F32R)
    with tc.tile_pool(name="bcpool", bufs=1) as bcpool:
        bcast_sb = bcpool.tile([128, S_], F32)
        nc.sync.dma_start(out=bcast_sb, in_=scale_d.to_broadcast([128, S_]))
        for c in range(CT):
            nc.vector.tensor_mul(xTr[:, c, :], xTr[:, c, :], bcast_sb)

    # --- main loop (phase 2) ---
    ptpool = stack.enter_context(tc.tile_pool(name="ptpool", bufs=7))
    spool = stack.enter_context(tc.tile_pool(name="spool", bufs=2))
    simsb = stack.enter_context(tc.tile_pool(name="simsb", bufs=2))
    smallpool = stack.enter_context(tc.tile_pool(name="smallpool", bufs=2))
    mpool = (stack.enter_context(tc.tile_pool(name="mpool", bufs=2))
             if mask_d is not None else None)

    with tc.tile_pool(name="simpsum", bufs=2, space="PSUM") as simp, \
         tc.tile_pool(name="scpsum", bufs=1, space="PSUM") as scp, \
         tc.tile_pool(name="ctxpsum", bufs=1, space="PSUM") as ctxp:

        def emit_ctx(ctx_ps, kt, pt_pairs):
            for j in range(PAIRS):
                for hi in range(2):
                    nc.tensor.matmul(
                        ctx_ps[2 * j + hi],
                        v2_sb[:, 2 * j + hi, kt, :],
                        pt_pairs[j][:, hi, :],
                        start=(kt == 0),
                        stop=(kt == NKT - 1),
                        skip_group_check=True,
                    )

        def emit_division_head(qb, ctx_ps, h):
            j, hi = divmod(h, 2)
            r0 = smallpool.tile([1, QB], F32, tag=f"r0{h % 2}",
                                name=f"r0_{qb}_{h}")
            nc.vector.reciprocal(
                r0, ctx_ps[h][HEAD_DIM:HEAD_DIM + 1, :])
            rb = smallpool.tile([HEAD_DIM, QB], F32, tag="rb")
            nc.gpsimd.partition_broadcast(rb, r0, channels=HEAD_DIM)
            nc.vector.tensor_mul(
                ctxT2_sb[hi * 64:hi * 64 + 64, j, qb * QB:(qb + 1) * QB],
                ctx_ps[h][0:HEAD_DIM, :],
                rb,
            )

        def emit_division(qb, ctx_ps):
            for h in range(HPC_):
                emit_division_head(qb, ctx_ps, h)

        prev_div = None
        for qb in range(NQB):
            ctx_ps = [ctxp.tile([HEAD_DIM + 1, QB], F32, tag=f"ctx{h}",
                                name=f"ctx_{qb}_{h}")
                      for h in range(HPC_)]
            pending = []
            for kt in range(NKT):
                if prev_div is not None and kt >= 2 and (kt - 2) % 3 == 0:
                    h = (kt - 2) // 3
                    if h < HPC_:
                        emit_division_head(prev_div[0], prev_div[1], h)
                        if h == HPC_ - 1:
                            prev_div = None
                sp = simp.tile([128, QB], F32, tag="sim")
                for c in range(CT):
                    nc.tensor.matmul(
                        sp,
                        xTr[:, c, kt * 128:(kt + 1) * 128],
                        xTr[:, c, qb * QB:(qb + 1) * QB],
                        start=(c == 0),
                        stop=(c == CT - 1),
                    )
                sim_t = simsb.tile([128, QB], F32, tag="simsb")
                nc.scalar.activation(out=sim_t, in_=sp, func=ACT_COPY)
                if mask_d is not None:
                    m_sb = mpool.tile([128, QB], F32, tag="msk")
                    nc.sync.dma_start(
                        out=m_sb,
                        in_=mask_d[kt * 128:(kt + 1) * 128, qb * QB:(qb + 1) * QB],
                    )
                    nc.vector.tensor_sub(sim_t, sim_t, m_sb)
                pt_pairs = []
                for j in range(PAIRS):
                    sc_t = scp.tile([128, 2, QB], F32, tag="scp")
                    for hi in range(2):
                        pr = slice(hi * 64, hi * 64 + 64)
                        nc.tensor.matmul(
                            sc_t[:, hi, :],
                            kT_sb[pr, j, kt * 128:(kt + 1) * 128],
                            qT_sb[pr, j, qb * QB:(qb + 1) * QB],
                            start=True,
                            stop=True,
                        )
                    s_t = spool.tile([128, 2, QB], F32, tag="s")
                    nc.vector.tensor_sub(
                        s_t, sc_t,
                        sim_t.unsqueeze(1).to_broadcast([128, 2, QB]))
                    pt = ptpool.tile([128, 2, QB], F32R, tag="pt")
                    nc.scalar.activation(out=pt, in_=s_t, func=ACT_EXP)
                    pt_pairs.append(pt)
                pending.append((kt, pt_pairs))
                if len(pending) > LAG:
                    k0, p0 = pending.pop(0)
                    emit_ctx(ctx_ps, k0, p0)
            for k0, p0 in pending:
                emit_ctx(ctx_ps, k0, p0)
            if prev_div is not None:
                done = max(0, (NKT - 1 - 2) // 3 + 1) if NKT > 2 else 0
                for h in range(min(done, HPC_), HPC_):
                    emit_division_head(prev_div[0], prev_div[1], h)
                prev_div = None
            prev_div = (qb, ctx_ps)
        emit_division(*prev_div)

    # --- out-projection (phase 3) ---
    with tc.tile_pool(name="outpsum", bufs=4, space="PSUM") as outp, \
         tc.tile_pool(name="outstg", bufs=4) as outstg:
        for qt in range(S_ // 128):
            for ob in range(NOB):
                op = outp.tile([128, OB_W], F32, tag="op")
                for j in range(PAIRS):
                    nc.tensor.matmul(
                        op,
                        ctxT2_sb[:, j, qt * 128:(qt + 1) * 128],
                        wo_sb[:, j, ob * OB_W:(ob + 1) * OB_W],
                        start=(j == 0),
                        stop=(j == PAIRS - 1),
                    )
                ostg = outstg.tile([128, OB_W], F32, tag="ostg")
                if (qt + ob) % 2 == 0:
                    nc.scalar.activation(out=ostg, in_=op, func=ACT_COPY)
                else:
                    nc.vector.tensor_copy(ostg, op)
                nc.sync.dma_start(
                    out=out_d[qt * 128:(qt + 1) * 128, ob * OB_W:(ob + 1) * OB_W],
                    in_=ostg,
                )

    stack.close()


def D2_of(hpc):
    return hpc * HEAD_DIM


def build_nc(*, S_=S, C_=HIDDEN, HPC_=HPC, QB=512, with_mask=False,
             enable_asserts=False):
    nc = bacc.Bacc(
        "TRN2", target_bir_lowering=False, debug=False,
        enable_asserts=enable_asserts,
    )
    D2 = HPC_ * HEAD_DIM
    aps = {}
    aps["xT"] = nc.dram_tensor("xT", [C_, S_], F32, kind="ExternalInput").ap()
    aps["scale"] = nc.dram_tensor("scale", [1, S_], F32, kind="ExternalInput").ap()
    for n in ("wq", "wk", "wv"):
        aps[n] = nc.dram_tensor(n, [C_, D2], F32, kind="ExternalInput").ap()
    aps["wo"] = nc.dram_tensor("wo", [D2, C_], F32, kind="ExternalInput").ap()
    for n in ("bq", "bk", "bv"):
        aps[n] = nc.dram_tensor(n, [D2, 1], F32, kind="ExternalInput").ap()
    if with_mask:
        aps["maskadd"] = nc.dram_tensor(
            "maskadd", [S_, S_], F32, kind="ExternalInput").ap()
    aps["out"] = nc.dram_tensor("out", [S_, C_], F32, kind="ExternalOutput").ap()

    with tile.TileContext(nc) as tc:
        emit_kernel(tc, aps, S_=S_, C_=C_, HPC_=HPC_, QB=QB)
    nc.compile()
    return nc


def host_prepare(x, attn_mask, Wq, bq, Wk, bk, Wv, bv, Wo, bo, *,
                 S_=S, C_=HIDDEN, HPC_=HPC, n_cores=N_CORES):
    """Build the per-core input maps. Returns (in_maps, with_mask)."""
    x = np.asarray(x, np.float32)
    B_ = x.shape[0]
    groups = n_cores // B_
    Wq = np.asarray(Wq, np.float32); Wk = np.asarray(Wk, np.float32)
    Wv = np.asarray(Wv, np.float32); Wo = np.asarray(Wo, np.float32)
    bq = np.asarray(bq, np.float32); bk = np.asarray(bk, np.float32)
    bv = np.asarray(bv, np.float32)

    inv_sqrt_d = 1.0 / math.sqrt(HEAD_DIM)
    WqT = np.ascontiguousarray((Wq * inv_sqrt_d).T)  # [C, C] in->out
    WkT = np.ascontiguousarray(Wk.T)
    WvT = np.ascontiguousarray(Wv.T)
    WoT = np.ascontiguousarray(Wo.T)                 # [C(c), C(o)]
    bq = bq * inv_sqrt_d

    mask = np.asarray(attn_mask)
    with_mask = bool(mask.any())
    maskadd = None
    if with_mask:
        # reference: where(mask, -inf); use a large negative additive bias
        maskadd = np.where(mask, np.float32(-1e30), np.float32(0.0)).astype(np.float32)
        # device layout: maskadd[k, q] added to scores^T
        maskadd = np.ascontiguousarray(maskadd.T)  # [k, q] = mask[q, k].T

    in_maps = []
    for core in range(n_cores):
        b, g = divmod(core, groups)
        xb = x[b]                                   # [S, C]
        xT = np.ascontiguousarray(xb.T)             # [C, S]
        norms = np.linalg.norm(xb, axis=1)          # [S]
        scale = (math.sqrt(GAMMA) / np.maximum(norms, 1e-12)).astype(np.float32)
        ch = slice(g * HPC_ * HEAD_DIM, (g + 1) * HPC_ * HEAD_DIM)
        m = {
            "xT": xT,
            "scale": scale.reshape(1, S_),
            "wq": np.ascontiguousarray(WqT[:, ch]),
            "wk": np.ascontiguousarray(WkT[:, ch]),
            "wv": np.ascontiguousarray(WvT[:, ch]),
            "wo": np.ascontiguousarray(WoT[ch, :]),
            "bq": np.ascontiguousarray(bq[ch]).reshape(-1, 1),
            "bk": np.ascontiguousarray(bk[ch]).reshape(-1, 1),
            "bv": np.ascontiguousarray(bv[ch]).reshape(-1, 1),
        }
        if with_mask:
            m["maskadd"] = maskadd
        in_maps.append(m)
    return in_maps, with_mask


_NC_CACHE = {}


def _get_nc(with_mask):
    key = with_mask
    if key not in _NC_CACHE:
        _NC_CACHE[key] = build_nc(with_mask=with_mask)
    return _NC_CACHE[key]


LAST_RESULTS = None


def kernel(**inputs):
    global LAST_RESULTS
    in_maps, with_mask = host_prepare(
        inputs["x"], inputs["attn_mask"],
        inputs["Wq"], inputs["bq"], inputs["Wk"], inputs["bk"],
        inputs["Wv"], inputs["bv"], inputs["Wo"], inputs["bo"],
    )
    nc = _get_nc(with_mask)
    res = run_bass_kernel_spmd(nc, in_maps, core_ids=list(range(N_CORES)))
    LAST_RESULTS = res
    bo = np.asarray(inputs["bo"], np.float32)
    out = np.zeros((B, S, HIDDEN), np.float32)
    groups = N_CORES // B
    for core in range(N_CORES):
        b = core // groups
        out[b] += res.results[core]["out"]
    out += bo[None, None, :]
    return out



# revision 9
# speedup vs baseline: 1.3900x; 1.3900x over previous
"""DiversityAttention on 8 TRN2 NeuronCores (Bass/Tile), bf16 PE path.

Sharding: data-parallel over batch (B=2) x tensor-parallel over heads
(16 heads -> 4 groups of 4). core = (b, g), b = core // 4, g = core % 4.
Each core computes full attention for its 4 heads over its batch and a
partial out-projection [S, HIDDEN]; the host sums the 4 partials per
batch and adds bo.

Everything on the PE runs bf16 (1 col/cycle streaming; fp32r measured
at ~half rate on HW), accumulating in f32 PSUM. Host pre-casts inputs
to bf16 and pre-normalizes x for the sim term.

Device formulation, keys-on-partitions ("S^T") orientation:
  qT = (Wq/8 @ xb + bq/8)  [128(2h*64), pair, S]   bf16
  kT = (Wk @ xb + bk)      likewise
  vT -> PE-transpose -> V [keys, h, kt, 64]        bf16 (no ones col)
  per (qb, kt):
    sim_ps  = xh^T xh (raw cosine)                 psum f32
    E       = exp(-gamma * sim_ps)   (ACT, scale=-gamma) -> bf16
    sc_ps_j = kT^T qT (row-tiled pair: 2 concurrent K=64 matmuls)
    pexp_j  = exp(sc_ps_j)           (ACT, straight from PSUM) -> bf16
    pt_j    = pexp_j * E             (DVE 2x bf16)
    ctx_j  += V^T pt   (col-tiled M=64 pair: 2 concurrent matmuls)
    sums   += ones^T pt (4 col-tiled M=1 matmuls at cols 0/32/64/96)
  division: one reciprocal over the 4 strided sums rows, gpsimd
  partition-broadcast, DVE mul -> ctxT2 bf16; out-projection of the
  previous query block is interleaved into the current block's loop.
"""

import math
import os
import sys

import numpy as np

for _p in ("/opt/trn_rl_repo",):
    if _p not in sys.path and os.path.isdir(_p):
        sys.path.insert(0, _p)

os.environ.setdefault("MYCRO_LOCAL_CACHE", "1")

import ml_dtypes

import concourse.bass as bass
import concourse.tile as tile
from concourse import bacc, mybir
from concourse.bass_utils import run_bass_kernel_spmd
from concourse.masks import make_identity


def _install_ntff_hook():
    """Provide antenv.axon_hooks (NTFF profiling registry) if the image
    lacks it, mirroring trn_agent_boot's ctypes hook. No-op on failure."""
    try:
        import antenv.axon_hooks  # noqa: F401
        return
    except ImportError:
        pass
    try:
        import contextlib
        import ctypes
        import types

        so_path = "/opt/axon/libaxon_pjrt.so"
        if not os.path.exists(so_path):
            return
        lib = ctypes.CDLL(so_path)
        if not hasattr(lib, "axon_start_nrt_profile"):
            return
        lib.axon_start_nrt_profile.argtypes = [
            ctypes.POINTER(ctypes.c_int64), ctypes.c_size_t]
        lib.axon_start_nrt_profile.restype = ctypes.c_int64
        lib.axon_stop_nrt_profile.argtypes = [ctypes.c_char_p]
        lib.axon_stop_nrt_profile.restype = ctypes.c_int64

        @contextlib.contextmanager
        def _hook(output_dir, device_ids):
            import jax
            jax.devices()
            if device_ids:
                ids = (ctypes.c_int64 * len(device_ids))(*device_ids)
                rc = lib.axon_start_nrt_profile(ids, len(device_ids))
            else:
                rc = lib.axon_start_nrt_profile(None, 0)
            if rc != 0:
                raise RuntimeError(f"axon_start_nrt_profile rc={rc}")
            try:
                yield
            finally:
                n = lib.axon_stop_nrt_profile(str(output_dir).encode())
                print(f"ntff profile: {n} file(s) -> {output_dir}",
                      file=sys.stderr)

        mod = types.ModuleType("antenv.axon_hooks")
        _state = {"hook": _hook}
        mod.set_axon_ntff_profile_hook = lambda h: _state.__setitem__("hook", h)
        mod.get_axon_ntff_profile_hook = lambda: _state["hook"]
        sys.modules["antenv.axon_hooks"] = mod
        import antenv
        antenv.axon_hooks = mod
    except Exception:
        pass


_install_ntff_hook()

F32 = mybir.dt.float32
BF16 = mybir.dt.bfloat16
ACT_EXP = mybir.ActivationFunctionType.Exp
ACT_COPY = mybir.ActivationFunctionType.Copy
ACT_IDENT = mybir.ActivationFunctionType.Identity
ALU = mybir.AluOpType

# Problem constants (hardcoded per contract).
HIDDEN = 1024
HEADS = 16
HEAD_DIM = 64
GAMMA = 0.5
B, S = 2, 2048
N_CORES = 8
GROUPS = N_CORES // B   # head groups per batch
HPC = HEADS // GROUPS   # heads per core
PAIRS = HPC // 2
CT = HIDDEN // 128      # contraction tiles
QB = 512
NQB = S // QB
NKT = S // 128
LAG = 2                 # kt lag between pt and ctx matmul
MASK_BIG = 60.0         # additive mask magnitude inside exp


def emit_kernel(tc, aps):
    nc = tc.nc

    xb_d = aps["xb"]; xh_d = aps["xh"]
    wq_d = aps["wq"]; wk_d = aps["wk"]; wv_d = aps["wv"]; wo_d = aps["wo"]
    bq_d = aps["bq"]; bk_d = aps["bk"]; bv_d = aps["bv"]
    out_d = aps["out"]
    mask_d = aps.get("maskadd")

    from contextlib import ExitStack
    stack = ExitStack()
    consts = stack.enter_context(tc.tile_pool(name="consts", bufs=1))

    identity = consts.tile([128, 128], BF16)
    make_identity(nc, identity)
    ones_sb = consts.tile([128, 1], BF16)
    nc.vector.memset(ones_sb, 1.0)
    ones64 = consts.tile([128, 64], F32)
    nc.vector.memset(ones64, 1.0)

    xb_sb = consts.tile([128, CT, S], BF16)
    xh_sb = consts.tile([128, CT, S], BF16)
    wq_sb = consts.tile([128, CT, 2 * 128], BF16)
    wk_sb = consts.tile([128, CT, 2 * 128], BF16)
    wv_sb = consts.tile([128, CT, 2 * 128], BF16)
    wo_sb = consts.tile([128, PAIRS, HIDDEN], BF16)
    bq_sb = consts.tile([128, PAIRS, 1], F32)
    bk_sb = consts.tile([128, PAIRS, 1], F32)
    bv_sb = consts.tile([128, PAIRS, 1], F32)

    qT = consts.tile([128, PAIRS, S], BF16)
    kT = consts.tile([128, PAIRS, S], BF16)
    v2 = consts.tile([128, HPC, NKT, HEAD_DIM], BF16)
    ctxT2 = consts.tile([128, PAIRS, S], BF16)

    # ---- loads (ordered so proj-q can start earliest) ----
    nc.sync.dma_start(out=wq_sb, in_=wq_d.rearrange("(t p) m -> p t m", p=128))
    nc.sync.dma_start(out=bq_sb, in_=bq_d.rearrange("(j p) one -> p j one", p=128))
    nc.sync.dma_start(out=xb_sb, in_=xb_d.rearrange("(t p) m -> p t m", p=128))
    nc.sync.dma_start(out=wk_sb, in_=wk_d.rearrange("(t p) m -> p t m", p=128))
    nc.sync.dma_start(out=bk_sb, in_=bk_d.rearrange("(j p) one -> p j one", p=128))
    nc.sync.dma_start(out=wv_sb, in_=wv_d.rearrange("(t p) m -> p t m", p=128))
    nc.sync.dma_start(out=bv_sb, in_=bv_d.rearrange("(j p) one -> p j one", p=128))
    nc.sync.dma_start(out=xh_sb, in_=xh_d.rearrange("(t p) m -> p t m", p=128))
    nc.sync.dma_start(out=wo_sb, in_=wo_d.rearrange("(j p) o -> p j o", p=128))

    # ---- phase 1: projections ----
    with tc.tile_pool(name="vtmp", bufs=1) as vtmp, \
         tc.tile_pool(name="projps", bufs=2, space="PSUM") as projps, \
         tc.tile_pool(name="tpps", bufs=2, space="PSUM") as tpps:
        vT = vtmp.tile([128, PAIRS, S], BF16)
        for w_sb, b_sb, dest in ((wq_sb, bq_sb, qT), (wk_sb, bk_sb, kT),
                                 (wv_sb, bv_sb, vT)):
            for j in range(PAIRS):
                for nb in range(S // QB):
                    ps = projps.tile([128, QB], F32, tag="prj",
                                     name=f"prj_{dest.tensor.name}_{j}_{nb}")
                    for c in range(CT):
                        nc.tensor.matmul(
                            ps,
                            w_sb[:, c, j * 128:(j + 1) * 128],
                            xb_sb[:, c, nb * QB:(nb + 1) * QB],
                            start=(c == 0),
                            stop=(c == CT - 1),
                        )
                    nc.scalar.activation(
                        out=dest[:, j, nb * QB:(nb + 1) * QB], in_=ps,
                        func=ACT_IDENT, bias=b_sb[:, j, :])

        # V transpose: [dims, keys] -> [keys, h, kt, dim]
        for j in range(PAIRS):
            for t in range(NKT):
                tp = tpps.tile([128, 128], BF16, tag="tp")
                nc.tensor.transpose(tp, vT[:, j, t * 128:(t + 1) * 128],
                                    identity)
                nc.scalar.activation(
                    out=v2[:, 2 * j:2 * j + 2, t, :],
                    in_=tp.rearrange("p (h d) -> p h d", h=2),
                    func=ACT_COPY)

    # ---- phase 2: attention main loop ----
    simp = stack.enter_context(tc.tile_pool(name="simp", bufs=2, space="PSUM"))
    scp = stack.enter_context(tc.tile_pool(name="scp", bufs=1, space="PSUM"))
    ctxp = stack.enter_context(tc.tile_pool(name="ctxp", bufs=1, space="PSUM"))
    sumsp = stack.enter_context(tc.tile_pool(name="sumsp", bufs=1, space="PSUM"))
    outp = stack.enter_context(tc.tile_pool(name="outp", bufs=1, space="PSUM"))

    ep = stack.enter_context(tc.tile_pool(name="ep", bufs=3))
    pexpp = stack.enter_context(tc.tile_pool(name="pexpp", bufs=3))
    ptp = stack.enter_context(tc.tile_pool(name="ptp", bufs=8))
    stagep = stack.enter_context(tc.tile_pool(name="stagep", bufs=3))
    r0p = stack.enter_context(tc.tile_pool(name="r0p", bufs=2))
    rbp = stack.enter_context(tc.tile_pool(name="rbp", bufs=4))
    mp = (stack.enter_context(tc.tile_pool(name="mp", bufs=2))
          if mask_d is not None else None)
    msp = (stack.enter_context(tc.tile_pool(name="msp", bufs=2))
           if mask_d is not None else None)

    def emit_ctx(ctx, sums, kt, pts):
        for j in range(PAIRS):
            for hi in range(2):
                nc.tensor.matmul(
                    ctx[j][64 * hi:64 * hi + 64, :],
                    v2[:, 2 * j + hi, kt, :],
                    pts[j][:, hi, :],
                    start=(kt == 0),
                    stop=(kt == NKT - 1),
                    skip_group_check=True,
                )
        for h in range(HPC):
            j, hi = divmod(h, 2)
            nc.tensor.matmul(
                sums[32 * h:32 * h + 1, :],
                ones_sb,
                pts[j][:, hi, :],
                start=(kt == 0),
                stop=(kt == NKT - 1),
                tile_position=(0, 32 * h),
                skip_group_check=True,
            )

    def emit_recip(qb0, sums):
        # reciprocal over all 97 partitions (DVE cost ~ free-dim only);
        # only rows 0/32/64/96 hold real sums, the rest is junk never read.
        r0 = r0p.tile([97, QB], F32, tag="r0", name=f"r0_{qb0}")
        nc.vector.reciprocal(r0, sums)
        return r0

    def emit_divmuls(qb0, ctx, r0):
        # broadcast r0 rows across partitions via K=1 outer product with
        # ones (gpsimd partition_broadcast mishandles offset APs on HW).
        for j in range(PAIRS):
            rb_ps = simp.tile([128, QB], F32, tag="sim",
                              name=f"rbps_{qb0}_{j}")
            for hi in range(2):
                h = 2 * j + hi
                nc.tensor.matmul(
                    rb_ps[64 * hi:64 * hi + 64, :],
                    ones64[32 * h:32 * h + 1, :],
                    r0[32 * h:32 * h + 1, :],
                    start=True, stop=True,
                    tile_position=(32 * h, 64 * hi),
                )
            rb = rbp.tile([128, QB], F32, tag="rb", name=f"rb_{qb0}_{j}")
            nc.vector.tensor_copy(rb, rb_ps)
            nc.vector.tensor_mul(ctxT2[:, j, qb0 * QB:(qb0 + 1) * QB],
                                 ctx[j], rb)

    def emit_outproj_tile(qb0, i):
        qt = qb0 * (QB // 128) + i // 2
        ob = i % 2
        op = outp.tile([128, 512], F32, tag="op", name=f"op_{qb0}_{i}")
        for j in range(PAIRS):
            nc.tensor.matmul(
                op,
                ctxT2[:, j, qt * 128:(qt + 1) * 128],
                wo_sb[:, j, ob * 512:(ob + 1) * 512],
                start=(j == 0),
                stop=(j == PAIRS - 1),
            )
        st = stagep.tile([128, 512], F32, tag="st", name=f"st_{qb0}_{i}")
        if i % 2 == 0:
            nc.vector.tensor_copy(st, op)
        else:
            nc.scalar.activation(out=st, in_=op, func=ACT_COPY)
        nc.sync.dma_start(
            out=out_d[qt * 128:(qt + 1) * 128, ob * 512:(ob + 1) * 512],
            in_=st)

    prev = None
    for qb in range(NQB):
        qsl = slice(qb * QB, (qb + 1) * QB)
        ctx = [ctxp.tile([128, QB], F32, tag=f"ctx{j}", name=f"ctx_{qb}_{j}")
               for j in range(PAIRS)]
        sums = sumsp.tile([97, QB], F32, tag="sums", name=f"sums_{qb}")
        nc.vector.memset(sums, 1.0)  # init junk rows for the [97,·] recip
        pending = []
        for kt in range(NKT):
            if prev is not None:
                if kt == 2:
                    emit_divmuls(*prev)
                elif 4 <= kt <= 11:
                    emit_outproj_tile(prev[0], kt - 4)
                if kt == NKT - 1:
                    prev = None
            ksl = slice(kt * 128, (kt + 1) * 128)
            # sim
            sp = simp.tile([128, QB], F32, tag="sim", name=f"sim_{qb}_{kt}")
            for c in range(CT):
                nc.tensor.matmul(sp, xh_sb[:, c, ksl], xh_sb[:, c, qsl],
                                 start=(c == 0), stop=(c == CT - 1))
            E = ep.tile([128, QB], BF16, tag="E", name=f"E_{qb}_{kt}")
            if mask_d is None:
                nc.scalar.activation(out=E, in_=sp, func=ACT_EXP, scale=-GAMMA)
            else:
                m_sb = mp.tile([128, QB], BF16, tag="m")
                nc.sync.dma_start(out=m_sb, in_=mask_d[ksl, qsl])
                ms = msp.tile([128, QB], BF16, tag="ms")
                nc.vector.scalar_tensor_tensor(
                    out=ms, in0=sp, scalar=-GAMMA, in1=m_sb,
                    op0=ALU.mult, op1=ALU.subtract)
                nc.scalar.activation(out=E, in_=ms, func=ACT_EXP)
            # scores + exp + pt per pair
            pts = []
            for j in range(PAIRS):
                sc = scp.tile([128, 2, QB], F32, tag="sc",
                              name=f"sc_{qb}_{kt}_{j}")
                for hi in range(2):
                    pr = slice(hi * 64, hi * 64 + 64)
                    nc.tensor.matmul(sc[:, hi, :], kT[pr, j, ksl],
                                     qT[pr, j, qsl], start=True, stop=True)
                pexp = pexpp.tile([128, 2, QB], BF16, tag="pexp",
                                  name=f"pexp_{qb}_{kt}_{j}")
                nc.scalar.activation(out=pexp, in_=sc, func=ACT_EXP)
                pt = ptp.tile([128, 2, QB], BF16, tag="pt",
                              name=f"pt_{qb}_{kt}_{j}")
                nc.vector.tensor_mul(
                    pt, pexp, E.unsqueeze(1).to_broadcast([128, 2, QB]))
                pts.append(pt)
            pending.append((kt, pts))
            if len(pending) > LAG:
                k0, p0 = pending.pop(0)
                emit_ctx(ctx, sums, k0, p0)
        for k0, p0 in pending:
            emit_ctx(ctx, sums, k0, p0)
        r0 = emit_recip(qb, sums)
        prev = (qb, ctx, r0)

    # tail: last block's division + out-projection
    emit_divmuls(*prev)
    for i in range(8):
        emit_outproj_tile(prev[0], i)

    stack.close()


def build_nc(*, with_mask=False, enable_asserts=False):
    nc = bacc.Bacc(
        "TRN2", target_bir_lowering=False, debug=False,
        enable_asserts=enable_asserts,
    )
    D2 = HPC * HEAD_DIM
    aps = {}
    aps["xb"] = nc.dram_tensor("xb", [HIDDEN, S], BF16, kind="ExternalInput").ap()
    aps["xh"] = nc.dram_tensor("xh", [HIDDEN, S], BF16, kind="ExternalInput").ap()
    for n in ("wq", "wk", "wv"):
        aps[n] = nc.dram_tensor(n, [HIDDEN, D2], BF16, kind="ExternalInput").ap()
    aps["wo"] = nc.dram_tensor("wo", [D2, HIDDEN], BF16, kind="ExternalInput").ap()
    for n in ("bq", "bk", "bv"):
        aps[n] = nc.dram_tensor(n, [D2, 1], F32, kind="ExternalInput").ap()
    if with_mask:
        aps["maskadd"] = nc.dram_tensor(
            "maskadd", [S, S], BF16, kind="ExternalInput").ap()
    aps["out"] = nc.dram_tensor("out", [S, HIDDEN], F32,
                                kind="ExternalOutput").ap()

    with tile.TileContext(nc) as tc:
        emit_kernel(tc, aps)
    nc.compile()
    return nc


def host_prepare(x, attn_mask, Wq, bq, Wk, bk, Wv, bv, Wo, bo):
    """Build the per-core input maps. Returns (in_maps, with_mask)."""
    x = np.asarray(x, np.float32)
    B_ = x.shape[0]
    groups = N_CORES // B_
    Wq = np.asarray(Wq, np.float32); Wk = np.asarray(Wk, np.float32)
    Wv = np.asarray(Wv, np.float32); Wo = np.asarray(Wo, np.float32)
    bq = np.asarray(bq, np.float32); bk = np.asarray(bk, np.float32)
    bv = np.asarray(bv, np.float32)

    inv_sqrt_d = np.float32(1.0 / math.sqrt(HEAD_DIM))
    bf = ml_dtypes.bfloat16
    WqT = np.ascontiguousarray((Wq * inv_sqrt_d).T.astype(bf))
    WkT = np.ascontiguousarray(Wk.T.astype(bf))
    WvT = np.ascontiguousarray(Wv.T.astype(bf))
    WoT = np.ascontiguousarray(Wo.T.astype(bf))
    bq = bq * inv_sqrt_d

    mask = np.asarray(attn_mask)
    with_mask = bool(mask.any())
    maskadd = None
    if with_mask:
        maskadd = np.ascontiguousarray(
            (mask.T.astype(np.float32) * MASK_BIG).astype(bf))

    in_maps = []
    per_batch = {}
    for b in range(B_):
        xbat = x[b]
        norms = np.linalg.norm(xbat, axis=1, keepdims=True)
        xhat = xbat / np.maximum(norms, 1e-12)
        per_batch[b] = (
            np.ascontiguousarray(xbat.T.astype(bf)),
            np.ascontiguousarray(xhat.T.astype(bf)),
        )
    for core in range(N_CORES):
        b, g = divmod(core, groups)
        xbT, xhT = per_batch[b]
        ch = slice(g * HPC * HEAD_DIM, (g + 1) * HPC * HEAD_DIM)
        m = {
            "xb": xbT,
            "xh": xhT,
            "wq": np.ascontiguousarray(WqT[:, ch]),
            "wk": np.ascontiguousarray(WkT[:, ch]),
            "wv": np.ascontiguousarray(WvT[:, ch]),
            "wo": np.ascontiguousarray(WoT[ch, :]),
            "bq": np.ascontiguousarray(bq[ch]).reshape(-1, 1),
            "bk": np.ascontiguousarray(bk[ch]).reshape(-1, 1),
            "bv": np.ascontiguousarray(bv[ch]).reshape(-1, 1),
        }
        if with_mask:
            m["maskadd"] = maskadd
        in_maps.append(m)
    return in_maps, with_mask


_NC_CACHE = {}


def _get_nc(with_mask):
    key = with_mask
    if key not in _NC_CACHE:
        _NC_CACHE[key] = build_nc(with_mask=with_mask)
    return _NC_CACHE[key]


LAST_RESULTS = None


def kernel(**inputs):
    global LAST_RESULTS
    in_maps, with_mask = host_prepare(
        inputs["x"], inputs["attn_mask"],
        inputs["Wq"], inputs["bq"], inputs["Wk"], inputs["bk"],
        inputs["Wv"], inputs["bv"], inputs["Wo"], inputs["bo"],
    )
    nc = _get_nc(with_mask)
    res = run_bass_kernel_spmd(nc, in_maps, core_ids=list(range(N_CORES)))
    LAST_RESULTS = res
    bo = np.asarray(inputs["bo"], np.float32)
    out = np.zeros((B, S, HIDDEN), np.float32)
    groups = N_CORES // B
    for core in range(N_CORES):
        b = core // groups
        out[b] += res.results[core]["out"]
    out += bo[None, None, :]
    return out


# revision 14
# speedup vs baseline: 1.4009x; 1.0079x over previous
"""DiversityAttention on 8 TRN2 NeuronCores (Bass/Tile), bf16 PE path.

Sharding: data-parallel over batch (B=2) x tensor-parallel over heads
(16 heads -> 4 groups of 4). core = (b, g), b = core // 4, g = core % 4.
Each core computes full attention for its 4 heads over its batch and a
partial out-projection [S, HIDDEN]; the host sums the 4 partials per
batch and adds bo.

Everything on the PE runs bf16 (1 col/cycle streaming; fp32r measured
at ~half rate on HW), accumulating in f32 PSUM. Host pre-casts inputs
to bf16 and pre-normalizes x for the sim term.

Device formulation, keys-on-partitions ("S^T") orientation:
  qT = (Wq/8 @ xb + bq/8)  [128(2h*64), pair, S]   bf16
  kT = (Wk @ xb + bk)      likewise
  vT -> PE-transpose -> V [keys, h, kt, 64]        bf16 (no ones col)
  per (qb, kt):
    sim_ps  = xh^T xh (raw cosine)                 psum f32
    E       = exp(-gamma * sim_ps)   (ACT, scale=-gamma) -> bf16
    sc_ps_j = kT^T qT (row-tiled pair: 2 concurrent K=64 matmuls)
    pexp_j  = exp(sc_ps_j)           (ACT, straight from PSUM) -> bf16
    pt_j    = pexp_j * E             (DVE 2x bf16)
    ctx_j  += V^T pt   (col-tiled M=64 pair: 2 concurrent matmuls)
    sums   += ones^T pt (4 col-tiled M=1 matmuls at cols 0/32/64/96)
  division: one reciprocal over the 4 strided sums rows, gpsimd
  partition-broadcast, DVE mul -> ctxT2 bf16; out-projection of the
  previous query block is interleaved into the current block's loop.
"""

import math
import os
import sys

import numpy as np

for _p in ("/opt/trn_rl_repo",):
    if _p not in sys.path and os.path.isdir(_p):
        sys.path.insert(0, _p)

os.environ.setdefault("MYCRO_LOCAL_CACHE", "1")

import ml_dtypes

import concourse.bass as bass
import concourse.tile as tile
from concourse import bacc, mybir
from concourse.bass_utils import run_bass_kernel_spmd
from concourse.masks import make_identity


def _install_ntff_hook():
    """Provide antenv.axon_hooks (NTFF profiling registry) if the image
    lacks it, mirroring trn_agent_boot's ctypes hook. No-op on failure."""
    try:
        import antenv.axon_hooks  # noqa: F401
        return
    except ImportError:
        pass
    try:
        import contextlib
        import ctypes
        import types

        so_path = "/opt/axon/libaxon_pjrt.so"
        if not os.path.exists(so_path):
            return
        lib = ctypes.CDLL(so_path)
        if not hasattr(lib, "axon_start_nrt_profile"):
            return
        lib.axon_start_nrt_profile.argtypes = [
            ctypes.POINTER(ctypes.c_int64), ctypes.c_size_t]
        lib.axon_start_nrt_profile.restype = ctypes.c_int64
        lib.axon_stop_nrt_profile.argtypes = [ctypes.c_char_p]
        lib.axon_stop_nrt_profile.restype = ctypes.c_int64

        @contextlib.contextmanager
        def _hook(output_dir, device_ids):
            import jax
            jax.devices()
            if device_ids:
                ids = (ctypes.c_int64 * len(device_ids))(*device_ids)
                rc = lib.axon_start_nrt_profile(ids, len(device_ids))
            else:
                rc = lib.axon_start_nrt_profile(None, 0)
            if rc != 0:
                raise RuntimeError(f"axon_start_nrt_profile rc={rc}")
            try:
                yield
            finally:
                n = lib.axon_stop_nrt_profile(str(output_dir).encode())
                print(f"ntff profile: {n} file(s) -> {output_dir}",
                      file=sys.stderr)

        mod = types.ModuleType("antenv.axon_hooks")
        _state = {"hook": _hook}
        mod.set_axon_ntff_profile_hook = lambda h: _state.__setitem__("hook", h)
        mod.get_axon_ntff_profile_hook = lambda: _state["hook"]
        sys.modules["antenv.axon_hooks"] = mod
        import antenv
        antenv.axon_hooks = mod
    except Exception:
        pass


_install_ntff_hook()

F32 = mybir.dt.float32
BF16 = mybir.dt.bfloat16
ACT_EXP = mybir.ActivationFunctionType.Exp
ACT_COPY = mybir.ActivationFunctionType.Copy
ACT_IDENT = mybir.ActivationFunctionType.Identity
ALU = mybir.AluOpType

# Problem constants (hardcoded per contract).
HIDDEN = 1024
HEADS = 16
HEAD_DIM = 64
GAMMA = 0.5
B, S = 2, 2048
N_CORES = 8
GROUPS = N_CORES // B   # head groups per batch
HPC = HEADS // GROUPS   # heads per core
PAIRS = HPC // 2
CT = HIDDEN // 128      # contraction tiles
QB = 512
NQB = S // QB
NKT = S // 128
LAG = 2                 # kt lag between pt and ctx matmul
MASK_BIG = 60.0         # additive mask magnitude inside exp


def emit_kernel(tc, aps):
    nc = tc.nc

    xb_d = aps["xb"]; xh_d = aps["xh"]
    wq_d = aps["wq"]; wk_d = aps["wk"]; wv_d = aps["wv"]; wo_d = aps["wo"]
    bq_d = aps["bq"]; bk_d = aps["bk"]; bv_d = aps["bv"]
    out_d = aps["out"]
    mask_d = aps.get("maskadd")

    from contextlib import ExitStack
    stack = ExitStack()
    consts = stack.enter_context(tc.tile_pool(name="consts", bufs=1))

    identity = consts.tile([128, 128], BF16)
    make_identity(nc, identity)
    ones_sb = consts.tile([128, 1], BF16)
    nc.vector.memset(ones_sb, 1.0)
    ones64 = consts.tile([128, 64], F32)
    nc.vector.memset(ones64, 1.0)

    xb_sb = consts.tile([128, CT, S], BF16)
    xh_sb = consts.tile([128, CT, S], BF16)
    wq_sb = consts.tile([128, CT, 2 * 128], BF16)
    wk_sb = consts.tile([128, CT, 2 * 128], BF16)
    wv_sb = consts.tile([128, CT, 2 * 128], BF16)
    wo_sb = consts.tile([128, PAIRS, HIDDEN], BF16)
    bq_sb = consts.tile([128, PAIRS, 1], F32)
    bk_sb = consts.tile([128, PAIRS, 1], F32)
    bv_sb = consts.tile([128, PAIRS, 1], F32)

    qT = consts.tile([128, PAIRS, S], BF16)
    kT = consts.tile([128, PAIRS, S], BF16)
    v2 = consts.tile([128, HPC, NKT, HEAD_DIM], BF16)
    ctxT2 = consts.tile([128, PAIRS, S], BF16)

    # ---- loads (q-proj pipelines with per-chunk xb DMA) ----
    nc.sync.dma_start(out=wq_sb, in_=wq_d.rearrange("(t p) m -> p t m", p=128))
    nc.sync.dma_start(out=bq_sb, in_=bq_d.rearrange("(j p) one -> p j one", p=128))
    xb_r = xb_d.rearrange("(t p) m -> p t m", p=128)
    for c in range(CT):
        nc.sync.dma_start(out=xb_sb[:, c, :], in_=xb_r[:, c, :])
    nc.sync.dma_start(out=wk_sb, in_=wk_d.rearrange("(t p) m -> p t m", p=128))
    nc.sync.dma_start(out=bk_sb, in_=bk_d.rearrange("(j p) one -> p j one", p=128))
    nc.sync.dma_start(out=wv_sb, in_=wv_d.rearrange("(t p) m -> p t m", p=128))
    nc.sync.dma_start(out=bv_sb, in_=bv_d.rearrange("(j p) one -> p j one", p=128))
    nc.sync.dma_start(out=xh_sb, in_=xh_d.rearrange("(t p) m -> p t m", p=128))
    nc.sync.dma_start(out=wo_sb, in_=wo_d.rearrange("(j p) o -> p j o", p=128))

    # ---- phase 1: projections ----
    # q-projection runs contraction-outer over 8 live psum tiles so each
    # matmul only needs one xb chunk -> overlaps the xb DMA.
    with tc.tile_pool(name="qprojps", bufs=1, space="PSUM") as qprojps:
        qps = [qprojps.tile([128, QB], F32, tag=f"qp{j}_{nb}",
                            name=f"qp_{j}_{nb}")
               for j in range(PAIRS) for nb in range(S // QB)]
        for c in range(CT):
            for j in range(PAIRS):
                for nb in range(S // QB):
                    nc.tensor.matmul(
                        qps[j * (S // QB) + nb],
                        wq_sb[:, c, j * 128:(j + 1) * 128],
                        xb_sb[:, c, nb * QB:(nb + 1) * QB],
                        start=(c == 0),
                        stop=(c == CT - 1),
                    )
        for j in range(PAIRS):
            for nb in range(S // QB):
                nc.scalar.activation(
                    out=qT[:, j, nb * QB:(nb + 1) * QB],
                    in_=qps[j * (S // QB) + nb],
                    func=ACT_IDENT, bias=bq_sb[:, j, :])

    with tc.tile_pool(name="vtmp", bufs=1) as vtmp, \
         tc.tile_pool(name="projps", bufs=2, space="PSUM") as projps, \
         tc.tile_pool(name="tpps", bufs=2, space="PSUM") as tpps:
        vT = vtmp.tile([128, PAIRS, S], BF16)
        for w_sb, b_sb, dest in ((wk_sb, bk_sb, kT), (wv_sb, bv_sb, vT)):
            for j in range(PAIRS):
                for nb in range(S // QB):
                    ps = projps.tile([128, QB], F32, tag="prj",
                                     name=f"prj_{dest.tensor.name}_{j}_{nb}")
                    for c in range(CT):
                        nc.tensor.matmul(
                            ps,
                            w_sb[:, c, j * 128:(j + 1) * 128],
                            xb_sb[:, c, nb * QB:(nb + 1) * QB],
                            start=(c == 0),
                            stop=(c == CT - 1),
                        )
                    nc.scalar.activation(
                        out=dest[:, j, nb * QB:(nb + 1) * QB], in_=ps,
                        func=ACT_IDENT, bias=b_sb[:, j, :])

        # V transpose: [dims, keys] -> [keys, h, kt, dim]
        for j in range(PAIRS):
            for t in range(NKT):
                tp = tpps.tile([128, 128], BF16, tag="tp")
                nc.tensor.transpose(tp, vT[:, j, t * 128:(t + 1) * 128],
                                    identity)
                nc.scalar.activation(
                    out=v2[:, 2 * j:2 * j + 2, t, :],
                    in_=tp.rearrange("p (h d) -> p h d", h=2),
                    func=ACT_COPY)

    # ---- phase 2: attention main loop ----
    simp = stack.enter_context(tc.tile_pool(name="simp", bufs=2, space="PSUM"))
    scp = stack.enter_context(tc.tile_pool(name="scp", bufs=1, space="PSUM"))
    ctxp = stack.enter_context(tc.tile_pool(name="ctxp", bufs=1, space="PSUM"))
    sumsp = stack.enter_context(tc.tile_pool(name="sumsp", bufs=1, space="PSUM"))
    outp = stack.enter_context(tc.tile_pool(name="outp", bufs=1, space="PSUM"))

    ep = stack.enter_context(tc.tile_pool(name="ep", bufs=3))
    pexpp = stack.enter_context(tc.tile_pool(name="pexpp", bufs=3))
    ptp = stack.enter_context(tc.tile_pool(name="ptp", bufs=10))
    stagep = stack.enter_context(tc.tile_pool(name="stagep", bufs=3))
    r0p = stack.enter_context(tc.tile_pool(name="r0p", bufs=2))
    rbp = stack.enter_context(tc.tile_pool(name="rbp", bufs=4))
    mp = (stack.enter_context(tc.tile_pool(name="mp", bufs=2))
          if mask_d is not None else None)
    msp = (stack.enter_context(tc.tile_pool(name="msp", bufs=2))
           if mask_d is not None else None)

    def emit_ctx_pair(ctx, sums, kt, j, pt):
        for hi in range(2):
            nc.tensor.matmul(
                ctx[j][64 * hi:64 * hi + 64, :],
                v2[:, 2 * j + hi, kt, :],
                pt[:, hi, :],
                start=(kt == 0),
                stop=(kt == NKT - 1),
                skip_group_check=True,
            )
        for hi in range(2):
            h = 2 * j + hi
            nc.tensor.matmul(
                sums[32 * h:32 * h + 1, :],
                ones_sb,
                pt[:, hi, :],
                start=(kt == 0),
                stop=(kt == NKT - 1),
                tile_position=(0, 32 * h),
                skip_group_check=True,
            )

    def emit_recip(qb0, sums):
        # reciprocal over all 97 partitions (DVE cost ~ free-dim only);
        # only rows 0/32/64/96 hold real sums, the rest is junk never read.
        r0 = r0p.tile([97, QB], F32, tag="r0", name=f"r0_{qb0}")
        nc.vector.reciprocal(r0, sums)
        return r0

    def emit_divmuls(qb0, ctx, r0):
        # broadcast r0 rows across partitions via K=1 outer product with
        # ones (gpsimd partition_broadcast mishandles offset APs on HW).
        for j in range(PAIRS):
            rb_ps = simp.tile([128, QB], F32, tag="sim",
                              name=f"rbps_{qb0}_{j}")
            for hi in range(2):
                h = 2 * j + hi
                nc.tensor.matmul(
                    rb_ps[64 * hi:64 * hi + 64, :],
                    ones64[32 * h:32 * h + 1, :],
                    r0[32 * h:32 * h + 1, :],
                    start=True, stop=True,
                    tile_position=(32 * h, 64 * hi),
                )
            rb = rbp.tile([128, QB], F32, tag="rb", name=f"rb_{qb0}_{j}")
            nc.vector.tensor_copy(rb, rb_ps)
            nc.vector.tensor_mul(ctxT2[:, j, qb0 * QB:(qb0 + 1) * QB],
                                 ctx[j], rb)

    def emit_outproj_tile(qb0, i):
        qt = qb0 * (QB // 128) + i // 2
        ob = i % 2
        op = outp.tile([128, 512], F32, tag="op", name=f"op_{qb0}_{i}")
        for j in range(PAIRS):
            nc.tensor.matmul(
                op,
                ctxT2[:, j, qt * 128:(qt + 1) * 128],
                wo_sb[:, j, ob * 512:(ob + 1) * 512],
                start=(j == 0),
                stop=(j == PAIRS - 1),
            )
        st = stagep.tile([128, 512], F32, tag="st", name=f"st_{qb0}_{i}")
        if i % 2 == 0:
            nc.vector.tensor_copy(st, op)
        else:
            nc.scalar.activation(out=st, in_=op, func=ACT_COPY)
        nc.sync.dma_start(
            out=out_d[qt * 128:(qt + 1) * 128, ob * 512:(ob + 1) * 512],
            in_=st)

    LAGP = 5  # pair-granular lag of ctx/sums emission behind pt production
    pending = []   # (qb, kt, j, pt)
    qstate = {}    # qb -> (ctx tiles, sums tile)
    recips = {}    # qb -> r0

    def get_qstate(qb0):
        if qb0 not in qstate:
            ctx0 = [ctxp.tile([128, QB], F32, tag=f"ctx{j}",
                              name=f"ctx_{qb0}_{j}")
                    for j in range(PAIRS)]
            sums0 = sumsp.tile([97, QB], F32, tag="sums", name=f"sums_{qb0}")
            nc.vector.memset(sums0, 1.0)  # init junk rows for [97,·] recip
            qstate[qb0] = (ctx0, sums0)
        return qstate[qb0]

    def pop_pending():
        qb0, kt0, j0, pt0 = pending.pop(0)
        ctx0, sums0 = get_qstate(qb0)
        emit_ctx_pair(ctx0, sums0, kt0, j0, pt0)
        if kt0 == NKT - 1 and j0 == PAIRS - 1:
            recips[qb0] = emit_recip(qb0, sums0)

    for qb in range(NQB):
        qsl = slice(qb * QB, (qb + 1) * QB)
        for kt in range(NKT):
            if kt == 2 and qb > 0:
                emit_divmuls(qb - 1, qstate[qb - 1][0], recips.pop(qb - 1))
            elif 3 <= kt <= 10 and qb > 0:
                emit_outproj_tile(qb - 1, kt - 3)
            if kt > 0:
                while len(pending) > LAGP:
                    pop_pending()
            ksl = slice(kt * 128, (kt + 1) * 128)
            # sim
            sp = simp.tile([128, QB], F32, tag="sim", name=f"sim_{qb}_{kt}")
            for c in range(CT):
                nc.tensor.matmul(sp, xh_sb[:, c, ksl], xh_sb[:, c, qsl],
                                 start=(c == 0), stop=(c == CT - 1))
            E = ep.tile([128, QB], BF16, tag="E", name=f"E_{qb}_{kt}")
            if mask_d is None:
                nc.scalar.activation(out=E, in_=sp, func=ACT_EXP, scale=-GAMMA)
            else:
                m_sb = mp.tile([128, QB], BF16, tag="m")
                nc.sync.dma_start(out=m_sb, in_=mask_d[ksl, qsl])
                ms = msp.tile([128, QB], BF16, tag="ms")
                nc.vector.scalar_tensor_tensor(
                    out=ms, in0=sp, scalar=-GAMMA, in1=m_sb,
                    op0=ALU.mult, op1=ALU.subtract)
                nc.scalar.activation(out=E, in_=ms, func=ACT_EXP)
            if kt == 0 and qb > 0:
                # drain the previous block behind sim(kt0)'s PE work
                while pending:
                    pop_pending()
            # scores + exp + pt per pair
            for j in range(PAIRS):
                sc = scp.tile([128, 2, QB], F32, tag="sc",
                              name=f"sc_{qb}_{kt}_{j}")
                for hi in range(2):
                    pr = slice(hi * 64, hi * 64 + 64)
                    nc.tensor.matmul(sc[:, hi, :], kT[pr, j, ksl],
                                     qT[pr, j, qsl], start=True, stop=True)
                if len(pending) > LAGP:
                    pop_pending()
                pexp = pexpp.tile([128, 2, QB], BF16, tag="pexp",
                                  name=f"pexp_{qb}_{kt}_{j}")
                nc.scalar.activation(out=pexp, in_=sc, func=ACT_EXP)
                pt = ptp.tile([128, 2, QB], BF16, tag="pt",
                              name=f"pt_{qb}_{kt}_{j}")
                nc.vector.tensor_mul(
                    pt, pexp, E.unsqueeze(1).to_broadcast([128, 2, QB]))
                get_qstate(qb)
                pending.append((qb, kt, j, pt))

    # tail: flush, then last block's division + out-projection
    while pending:
        pop_pending()
    emit_divmuls(NQB - 1, qstate[NQB - 1][0], recips.pop(NQB - 1))
    for i in range(8):
        emit_outproj_tile(NQB - 1, i)

    stack.close()


def build_nc(*, with_mask=False, enable_asserts=False):
    nc = bacc.Bacc(
        "TRN2", target_bir_lowering=False, debug=False,
        enable_asserts=enable_asserts,
    )
    D2 = HPC * HEAD_DIM
    aps = {}
    aps["xb"] = nc.dram_tensor("xb", [HIDDEN, S], BF16, kind="ExternalInput").ap()
    aps["xh"] = nc.dram_tensor("xh", [HIDDEN, S], BF16, kind="ExternalInput").ap()
    for n in ("wq", "wk", "wv"):
        aps[n] = nc.dram_tensor(n, [HIDDEN, D2], BF16, kind="ExternalInput").ap()
    aps["wo"] = nc.dram_tensor("wo", [D2, HIDDEN], BF16, kind="ExternalInput").ap()
    for n in ("bq", "bk", "bv"):
        aps[n] = nc.dram_tensor(n, [D2, 1], F32, kind="ExternalInput").ap()
    if with_mask:
        aps["maskadd"] = nc.dram_tensor(
            "maskadd", [S, S], BF16, kind="ExternalInput").ap()
    aps["out"] = nc.dram_tensor("out", [S, HIDDEN], F32,
                                kind="ExternalOutput").ap()

    with tile.TileContext(nc) as tc:
        emit_kernel(tc, aps)
    nc.compile()
    return nc


def host_prepare(x, attn_mask, Wq, bq, Wk, bk, Wv, bv, Wo, bo):
    """Build the per-core input maps. Returns (in_maps, with_mask)."""
    x = np.asarray(x, np.float32)
    B_ = x.shape[0]
    groups = N_CORES // B_
    Wq = np.asarray(Wq, np.float32); Wk = np.asarray(Wk, np.float32)
    Wv = np.asarray(Wv, np.float32); Wo = np.asarray(Wo, np.float32)
    bq = np.asarray(bq, np.float32); bk = np.asarray(bk, np.float32)
    bv = np.asarray(bv, np.float32)

    inv_sqrt_d = np.float32(1.0 / math.sqrt(HEAD_DIM))
    bf = ml_dtypes.bfloat16
    WqT = np.ascontiguousarray((Wq * inv_sqrt_d).T.astype(bf))
    WkT = np.ascontiguousarray(Wk.T.astype(bf))
    WvT = np.ascontiguousarray(Wv.T.astype(bf))
    WoT = np.ascontiguousarray(Wo.T.astype(bf))
    bq = bq * inv_sqrt_d

    mask = np.asarray(attn_mask)
    with_mask = bool(mask.any())
    maskadd = None
    if with_mask:
        maskadd = np.ascontiguousarray(
            (mask.T.astype(np.float32) * MASK_BIG).astype(bf))

    in_maps = []
    per_batch = {}
    for b in range(B_):
        xbat = x[b]
        norms = np.linalg.norm(xbat, axis=1, keepdims=True)
        xhat = xbat / np.maximum(norms, 1e-12)
        per_batch[b] = (
            np.ascontiguousarray(xbat.T.astype(bf)),
            np.ascontiguousarray(xhat.T.astype(bf)),
        )
    for core in range(N_CORES):
        b, g = divmod(core, groups)
        xbT, xhT = per_batch[b]
        ch = slice(g * HPC * HEAD_DIM, (g + 1) * HPC * HEAD_DIM)
        m = {
            "xb": xbT,
            "xh": xhT,
            "wq": np.ascontiguousarray(WqT[:, ch]),
            "wk": np.ascontiguousarray(WkT[:, ch]),
            "wv": np.ascontiguousarray(WvT[:, ch]),
            "wo": np.ascontiguousarray(WoT[ch, :]),
            "bq": np.ascontiguousarray(bq[ch]).reshape(-1, 1),
            "bk": np.ascontiguousarray(bk[ch]).reshape(-1, 1),
            "bv": np.ascontiguousarray(bv[ch]).reshape(-1, 1),
        }
        if with_mask:
            m["maskadd"] = maskadd
        in_maps.append(m)
    return in_maps, with_mask


_NC_CACHE = {}


def _get_nc(with_mask):
    key = with_mask
    if key not in _NC_CACHE:
        _NC_CACHE[key] = build_nc(with_mask=with_mask)
    return _NC_CACHE[key]


LAST_RESULTS = None


def kernel(**inputs):
    global LAST_RESULTS
    in_maps, with_mask = host_prepare(
        inputs["x"], inputs["attn_mask"],
        inputs["Wq"], inputs["bq"], inputs["Wk"], inputs["bk"],
        inputs["Wv"], inputs["bv"], inputs["Wo"], inputs["bo"],
    )
    nc = _get_nc(with_mask)
    res = run_bass_kernel_spmd(nc, in_maps, core_ids=list(range(N_CORES)))
    LAST_RESULTS = res
    bo = np.asarray(inputs["bo"], np.float32)
    out = np.zeros((B, S, HIDDEN), np.float32)
    groups = N_CORES // B
    for core in range(N_CORES):
        b = core // groups
        out[b] += res.results[core]["out"]
    out += bo[None, None, :]
    return out


# revision 15
# speedup vs baseline: 1.5355x; 1.0961x over previous
"""DiversityAttention on 8 TRN2 NeuronCores (Bass/Tile), bf16 PE path.

Sharding: data-parallel over batch (B=2) x tensor-parallel over heads
(16 heads -> 4 groups of 4). core = (b, g), b = core // 4, g = core % 4.
Each core computes full attention for its 4 heads over its batch and a
partial out-projection [S, HIDDEN]; the host sums the 4 partials per
batch and adds bo.

Everything on the PE runs bf16 (1 col/cycle streaming; fp32r measured
at ~half rate on HW), accumulating in f32 PSUM. Host pre-casts inputs
to bf16 and pre-normalizes x for the sim term.

Device formulation, keys-on-partitions ("S^T") orientation:
  qT = (Wq/8 @ xb + bq/8)  [128(2h*64), pair, S]   bf16
  kT = (Wk @ xb + bk)      likewise
  vT -> PE-transpose -> V [keys, h, kt, 64]        bf16 (no ones col)
  per (qb, kt):
    sim_ps  = xh^T xh (raw cosine)                 psum f32
    E       = exp(-gamma * sim_ps)   (ACT, scale=-gamma) -> bf16
    sc_ps_j = kT^T qT (row-tiled pair: 2 concurrent K=64 matmuls)
    pexp_j  = exp(sc_ps_j)           (ACT, straight from PSUM) -> bf16
    pt_j    = pexp_j * E             (DVE 2x bf16)
    ctx_j  += V^T pt   (col-tiled M=64 pair: 2 concurrent matmuls)
    sums   += ones^T pt (4 col-tiled M=1 matmuls at cols 0/32/64/96)
  division: one reciprocal over the 4 strided sums rows, gpsimd
  partition-broadcast, DVE mul -> ctxT2 bf16; out-projection of the
  previous query block is interleaved into the current block's loop.
"""

import math
import os
import sys

import numpy as np

for _p in ("/opt/trn_rl_repo",):
    if _p not in sys.path and os.path.isdir(_p):
        sys.path.insert(0, _p)

os.environ.setdefault("MYCRO_LOCAL_CACHE", "1")

import ml_dtypes

import concourse.bass as bass
import concourse.tile as tile
from concourse import bacc, mybir
from concourse.bass_utils import run_bass_kernel_spmd
from concourse.masks import make_identity


def _install_ntff_hook():
    """Provide antenv.axon_hooks (NTFF profiling registry) if the image
    lacks it, mirroring trn_agent_boot's ctypes hook. No-op on failure."""
    try:
        import antenv.axon_hooks  # noqa: F401
        return
    except ImportError:
        pass
    try:
        import contextlib
        import ctypes
        import types

        so_path = "/opt/axon/libaxon_pjrt.so"
        if not os.path.exists(so_path):
            return
        lib = ctypes.CDLL(so_path)
        if not hasattr(lib, "axon_start_nrt_profile"):
            return
        lib.axon_start_nrt_profile.argtypes = [
            ctypes.POINTER(ctypes.c_int64), ctypes.c_size_t]
        lib.axon_start_nrt_profile.restype = ctypes.c_int64
        lib.axon_stop_nrt_profile.argtypes = [ctypes.c_char_p]
        lib.axon_stop_nrt_profile.restype = ctypes.c_int64

        @contextlib.contextmanager
        def _hook(output_dir, device_ids):
            import jax
            jax.devices()
            if device_ids:
                ids = (ctypes.c_int64 * len(device_ids))(*device_ids)
                rc = lib.axon_start_nrt_profile(ids, len(device_ids))
            else:
                rc = lib.axon_start_nrt_profile(None, 0)
            if rc != 0:
                raise RuntimeError(f"axon_start_nrt_profile rc={rc}")
            try:
                yield
            finally:
                n = lib.axon_stop_nrt_profile(str(output_dir).encode())
                print(f"ntff profile: {n} file(s) -> {output_dir}",
                      file=sys.stderr)

        mod = types.ModuleType("antenv.axon_hooks")
        _state = {"hook": _hook}
        mod.set_axon_ntff_profile_hook = lambda h: _state.__setitem__("hook", h)
        mod.get_axon_ntff_profile_hook = lambda: _state["hook"]
        sys.modules["antenv.axon_hooks"] = mod
        import antenv
        antenv.axon_hooks = mod
    except Exception:
        pass


_install_ntff_hook()

F32 = mybir.dt.float32
BF16 = mybir.dt.bfloat16
FP8 = mybir.dt.float8e4
XQ8_SCALE = 16.0
ACT_EXP = mybir.ActivationFunctionType.Exp
ACT_COPY = mybir.ActivationFunctionType.Copy
ACT_IDENT = mybir.ActivationFunctionType.Identity
ALU = mybir.AluOpType

# Problem constants (hardcoded per contract).
HIDDEN = 1024
HEADS = 16
HEAD_DIM = 64
GAMMA = 0.5
B, S = 2, 2048
N_CORES = 8
GROUPS = N_CORES // B   # head groups per batch
HPC = HEADS // GROUPS   # heads per core
PAIRS = HPC // 2
CT = HIDDEN // 128      # contraction tiles
QB = 512
NQB = S // QB
NKT = S // 128
LAG = 2                 # kt lag between pt and ctx matmul
MASK_BIG = 60.0         # additive mask magnitude inside exp


def emit_kernel(tc, aps):
    nc = tc.nc

    xb_d = aps["xb"]; xq8_d = aps["xq8"]
    wq_d = aps["wq"]; wk_d = aps["wk"]; wv_d = aps["wv"]; wo_d = aps["wo"]
    bq_d = aps["bq"]; bk_d = aps["bk"]; bv_d = aps["bv"]
    out_d = aps["out"]
    mask_d = aps.get("maskadd")

    from contextlib import ExitStack
    stack = ExitStack()
    consts = stack.enter_context(tc.tile_pool(name="consts", bufs=1))

    identity = consts.tile([128, 128], BF16)
    make_identity(nc, identity)
    ones_sb = consts.tile([128, 1], BF16)
    nc.vector.memset(ones_sb, 1.0)
    ones64 = consts.tile([128, 64], F32)
    nc.vector.memset(ones64, 1.0)

    xb_sb = consts.tile([128, CT, S], BF16)
    xq8_sb = consts.tile([128, CT // 2, 2, S], FP8)
    wq_sb = consts.tile([128, CT, 2 * 128], BF16)
    wk_sb = consts.tile([128, CT, 2 * 128], BF16)
    wv_sb = consts.tile([128, CT, 2 * 128], BF16)
    wo_sb = consts.tile([128, PAIRS, HIDDEN], BF16)
    bq_sb = consts.tile([128, PAIRS, 1], F32)
    bk_sb = consts.tile([128, PAIRS, 1], F32)
    bv_sb = consts.tile([128, PAIRS, 1], F32)

    qT = consts.tile([128, PAIRS, S], BF16)
    kT = consts.tile([128, PAIRS, S], BF16)
    v2 = consts.tile([128, HPC, NKT, HEAD_DIM], BF16)
    ctxT2 = consts.tile([128, PAIRS, S], BF16)

    # ---- loads (q-proj pipelines with per-chunk xb DMA) ----
    nc.sync.dma_start(out=wq_sb, in_=wq_d.rearrange("(t p) m -> p t m", p=128))
    nc.sync.dma_start(out=bq_sb, in_=bq_d.rearrange("(j p) one -> p j one", p=128))
    xb_r = xb_d.rearrange("(t p) m -> p t m", p=128)
    for c in range(CT):
        nc.sync.dma_start(out=xb_sb[:, c, :], in_=xb_r[:, c, :])
    nc.sync.dma_start(out=wk_sb, in_=wk_d.rearrange("(t p) m -> p t m", p=128))
    nc.sync.dma_start(out=bk_sb, in_=bk_d.rearrange("(j p) one -> p j one", p=128))
    nc.sync.dma_start(out=wv_sb, in_=wv_d.rearrange("(t p) m -> p t m", p=128))
    nc.sync.dma_start(out=bv_sb, in_=bv_d.rearrange("(j p) one -> p j one", p=128))
    nc.sync.dma_start(
        out=xq8_sb,
        in_=xq8_d.rearrange("(c two p) m -> p c two m", c=CT // 2, two=2))
    nc.sync.dma_start(out=wo_sb, in_=wo_d.rearrange("(j p) o -> p j o", p=128))

    # ---- phase 1: projections ----
    # q-projection runs contraction-outer over 8 live psum tiles so each
    # matmul only needs one xb chunk -> overlaps the xb DMA.
    with tc.tile_pool(name="qprojps", bufs=1, space="PSUM") as qprojps:
        qps = [qprojps.tile([128, QB], F32, tag=f"qp{j}_{nb}",
                            name=f"qp_{j}_{nb}")
               for j in range(PAIRS) for nb in range(S // QB)]
        for c in range(CT):
            for j in range(PAIRS):
                for nb in range(S // QB):
                    nc.tensor.matmul(
                        qps[j * (S // QB) + nb],
                        wq_sb[:, c, j * 128:(j + 1) * 128],
                        xb_sb[:, c, nb * QB:(nb + 1) * QB],
                        start=(c == 0),
                        stop=(c == CT - 1),
                    )
        for j in range(PAIRS):
            for nb in range(S // QB):
                nc.scalar.activation(
                    out=qT[:, j, nb * QB:(nb + 1) * QB],
                    in_=qps[j * (S // QB) + nb],
                    func=ACT_IDENT, bias=bq_sb[:, j, :])

    with tc.tile_pool(name="vtmp", bufs=1) as vtmp, \
         tc.tile_pool(name="projps", bufs=2, space="PSUM") as projps, \
         tc.tile_pool(name="tpps", bufs=2, space="PSUM") as tpps:
        vT = vtmp.tile([128, PAIRS, S], BF16)
        for w_sb, b_sb, dest in ((wk_sb, bk_sb, kT), (wv_sb, bv_sb, vT)):
            for j in range(PAIRS):
                for nb in range(S // QB):
                    ps = projps.tile([128, QB], F32, tag="prj",
                                     name=f"prj_{dest.tensor.name}_{j}_{nb}")
                    for c in range(CT):
                        nc.tensor.matmul(
                            ps,
                            w_sb[:, c, j * 128:(j + 1) * 128],
                            xb_sb[:, c, nb * QB:(nb + 1) * QB],
                            start=(c == 0),
                            stop=(c == CT - 1),
                        )
                    nc.scalar.activation(
                        out=dest[:, j, nb * QB:(nb + 1) * QB], in_=ps,
                        func=ACT_IDENT, bias=b_sb[:, j, :])

        # V transpose: [dims, keys] -> [keys, h, kt, dim]
        for j in range(PAIRS):
            for t in range(NKT):
                tp = tpps.tile([128, 128], BF16, tag="tp")
                nc.tensor.transpose(tp, vT[:, j, t * 128:(t + 1) * 128],
                                    identity)
                nc.scalar.activation(
                    out=v2[:, 2 * j:2 * j + 2, t, :],
                    in_=tp.rearrange("p (h d) -> p h d", h=2),
                    func=ACT_COPY)

    # ---- phase 2: attention main loop ----
    simp = stack.enter_context(tc.tile_pool(name="simp", bufs=2, space="PSUM"))
    scp = stack.enter_context(tc.tile_pool(name="scp", bufs=1, space="PSUM"))
    ctxp = stack.enter_context(tc.tile_pool(name="ctxp", bufs=1, space="PSUM"))
    sumsp = stack.enter_context(tc.tile_pool(name="sumsp", bufs=1, space="PSUM"))
    outp = stack.enter_context(tc.tile_pool(name="outp", bufs=1, space="PSUM"))

    ep = stack.enter_context(tc.tile_pool(name="ep", bufs=3))
    pexpp = stack.enter_context(tc.tile_pool(name="pexpp", bufs=3))
    ptp = stack.enter_context(tc.tile_pool(name="ptp", bufs=10))
    stagep = stack.enter_context(tc.tile_pool(name="stagep", bufs=3))
    r0p = stack.enter_context(tc.tile_pool(name="r0p", bufs=2))
    rbp = stack.enter_context(tc.tile_pool(name="rbp", bufs=4))
    mp = (stack.enter_context(tc.tile_pool(name="mp", bufs=2))
          if mask_d is not None else None)
    msp = (stack.enter_context(tc.tile_pool(name="msp", bufs=2))
           if mask_d is not None else None)

    def emit_ctx_pair(ctx, sums, kt, j, pt):
        for hi in range(2):
            nc.tensor.matmul(
                ctx[j][64 * hi:64 * hi + 64, :],
                v2[:, 2 * j + hi, kt, :],
                pt[:, hi, :],
                start=(kt == 0),
                stop=(kt == NKT - 1),
                skip_group_check=True,
            )
        for hi in range(2):
            h = 2 * j + hi
            nc.tensor.matmul(
                sums[32 * h:32 * h + 1, :],
                ones_sb,
                pt[:, hi, :],
                start=(kt == 0),
                stop=(kt == NKT - 1),
                tile_position=(0, 32 * h),
                skip_group_check=True,
            )

    def emit_recip(qb0, sums):
        # reciprocal over all 97 partitions (DVE cost ~ free-dim only);
        # only rows 0/32/64/96 hold real sums, the rest is junk never read.
        r0 = r0p.tile([97, QB], F32, tag="r0", name=f"r0_{qb0}")
        nc.vector.reciprocal(r0, sums)
        return r0

    def emit_divmuls(qb0, ctx, r0):
        # broadcast r0 rows across partitions via K=1 outer product with
        # ones (gpsimd partition_broadcast mishandles offset APs on HW).
        for j in range(PAIRS):
            rb_ps = simp.tile([128, QB], F32, tag="sim",
                              name=f"rbps_{qb0}_{j}")
            for hi in range(2):
                h = 2 * j + hi
                nc.tensor.matmul(
                    rb_ps[64 * hi:64 * hi + 64, :],
                    ones64[32 * h:32 * h + 1, :],
                    r0[32 * h:32 * h + 1, :],
                    start=True, stop=True,
                    tile_position=(32 * h, 64 * hi),
                )
            rb = rbp.tile([128, QB], F32, tag="rb", name=f"rb_{qb0}_{j}")
            nc.vector.tensor_copy(rb, rb_ps)
            nc.vector.tensor_mul(ctxT2[:, j, qb0 * QB:(qb0 + 1) * QB],
                                 ctx[j], rb)

    def emit_outproj_tile(qb0, i):
        qt = qb0 * (QB // 128) + i // 2
        ob = i % 2
        op = outp.tile([128, 512], F32, tag="op", name=f"op_{qb0}_{i}")
        for j in range(PAIRS):
            nc.tensor.matmul(
                op,
                ctxT2[:, j, qt * 128:(qt + 1) * 128],
                wo_sb[:, j, ob * 512:(ob + 1) * 512],
                start=(j == 0),
                stop=(j == PAIRS - 1),
            )
        st = stagep.tile([128, 512], F32, tag="st", name=f"st_{qb0}_{i}")
        if i % 2 == 0:
            nc.vector.tensor_copy(st, op)
        else:
            nc.scalar.activation(out=st, in_=op, func=ACT_COPY)
        nc.sync.dma_start(
            out=out_d[qt * 128:(qt + 1) * 128, ob * 512:(ob + 1) * 512],
            in_=st)

    LAGP = 5  # pair-granular lag of ctx/sums emission behind pt production
    pending = []   # (qb, kt, j, pt)
    qstate = {}    # qb -> (ctx tiles, sums tile)
    recips = {}    # qb -> r0

    def get_qstate(qb0):
        if qb0 not in qstate:
            ctx0 = [ctxp.tile([128, QB], F32, tag=f"ctx{j}",
                              name=f"ctx_{qb0}_{j}")
                    for j in range(PAIRS)]
            sums0 = sumsp.tile([97, QB], F32, tag="sums", name=f"sums_{qb0}")
            nc.vector.memset(sums0, 1.0)  # init junk rows for [97,·] recip
            qstate[qb0] = (ctx0, sums0)
        return qstate[qb0]

    def pop_pending():
        qb0, kt0, j0, pt0 = pending.pop(0)
        ctx0, sums0 = get_qstate(qb0)
        emit_ctx_pair(ctx0, sums0, kt0, j0, pt0)
        if kt0 == NKT - 1 and j0 == PAIRS - 1:
            recips[qb0] = emit_recip(qb0, sums0)

    for qb in range(NQB):
        qsl = slice(qb * QB, (qb + 1) * QB)
        for kt in range(NKT):
            if kt == 2 and qb > 0:
                emit_divmuls(qb - 1, qstate[qb - 1][0], recips.pop(qb - 1))
            elif 3 <= kt <= 10 and qb > 0:
                emit_outproj_tile(qb - 1, kt - 3)
            if kt > 0:
                while len(pending) > LAGP:
                    pop_pending()
            ksl = slice(kt * 128, (kt + 1) * 128)
            # sim
            sp = simp.tile([128, QB], F32, tag="sim", name=f"sim_{qb}_{kt}")
            for c in range(CT // 2):
                nc.tensor.matmul(sp, xq8_sb[:, c, :, ksl], xq8_sb[:, c, :, qsl],
                                 start=(c == 0), stop=(c == CT // 2 - 1),
                                 perf_mode=mybir.MatmulPerfMode.DoubleRow)
            E = ep.tile([128, QB], BF16, tag="E", name=f"E_{qb}_{kt}")
            if mask_d is None:
                nc.scalar.activation(out=E, in_=sp, func=ACT_EXP, scale=-GAMMA / XQ8_SCALE ** 2)
            else:
                m_sb = mp.tile([128, QB], BF16, tag="m")
                nc.sync.dma_start(out=m_sb, in_=mask_d[ksl, qsl])
                ms = msp.tile([128, QB], BF16, tag="ms")
                nc.vector.scalar_tensor_tensor(
                    out=ms, in0=sp, scalar=-GAMMA / XQ8_SCALE ** 2, in1=m_sb,
                    op0=ALU.mult, op1=ALU.subtract)
                nc.scalar.activation(out=E, in_=ms, func=ACT_EXP)
            if kt == 0 and qb > 0:
                # drain the previous block behind sim(kt0)'s PE work
                while pending:
                    pop_pending()
            # scores + exp + pt per pair
            for j in range(PAIRS):
                sc = scp.tile([128, 2, QB], F32, tag="sc",
                              name=f"sc_{qb}_{kt}_{j}")
                for hi in range(2):
                    pr = slice(hi * 64, hi * 64 + 64)
                    nc.tensor.matmul(sc[:, hi, :], kT[pr, j, ksl],
                                     qT[pr, j, qsl], start=True, stop=True)
                if len(pending) > LAGP:
                    pop_pending()
                pexp = pexpp.tile([128, 2, QB], BF16, tag="pexp",
                                  name=f"pexp_{qb}_{kt}_{j}")
                nc.scalar.activation(out=pexp, in_=sc, func=ACT_EXP)
                pt = ptp.tile([128, 2, QB], BF16, tag="pt",
                              name=f"pt_{qb}_{kt}_{j}")
                nc.vector.tensor_mul(
                    pt, pexp, E.unsqueeze(1).to_broadcast([128, 2, QB]))
                get_qstate(qb)
                pending.append((qb, kt, j, pt))

    # tail: flush, then last block's division + out-projection
    while pending:
        pop_pending()
    emit_divmuls(NQB - 1, qstate[NQB - 1][0], recips.pop(NQB - 1))
    for i in range(8):
        emit_outproj_tile(NQB - 1, i)

    stack.close()


def build_nc(*, with_mask=False, enable_asserts=False):
    nc = bacc.Bacc(
        "TRN2", target_bir_lowering=False, debug=False,
        enable_asserts=enable_asserts,
    )
    D2 = HPC * HEAD_DIM
    aps = {}
    aps["xb"] = nc.dram_tensor("xb", [HIDDEN, S], BF16, kind="ExternalInput").ap()
    aps["xq8"] = nc.dram_tensor("xq8", [HIDDEN, S], FP8,
                                kind="ExternalInput").ap()
    for n in ("wq", "wk", "wv"):
        aps[n] = nc.dram_tensor(n, [HIDDEN, D2], BF16, kind="ExternalInput").ap()
    aps["wo"] = nc.dram_tensor("wo", [D2, HIDDEN], BF16, kind="ExternalInput").ap()
    for n in ("bq", "bk", "bv"):
        aps[n] = nc.dram_tensor(n, [D2, 1], F32, kind="ExternalInput").ap()
    if with_mask:
        aps["maskadd"] = nc.dram_tensor(
            "maskadd", [S, S], BF16, kind="ExternalInput").ap()
    aps["out"] = nc.dram_tensor("out", [S, HIDDEN], F32,
                                kind="ExternalOutput").ap()

    with tile.TileContext(nc) as tc:
        emit_kernel(tc, aps)
    nc.compile()
    return nc


def host_prepare(x, attn_mask, Wq, bq, Wk, bk, Wv, bv, Wo, bo):
    """Build the per-core input maps. Returns (in_maps, with_mask)."""
    x = np.asarray(x, np.float32)
    B_ = x.shape[0]
    groups = N_CORES // B_
    Wq = np.asarray(Wq, np.float32); Wk = np.asarray(Wk, np.float32)
    Wv = np.asarray(Wv, np.float32); Wo = np.asarray(Wo, np.float32)
    bq = np.asarray(bq, np.float32); bk = np.asarray(bk, np.float32)
    bv = np.asarray(bv, np.float32)

    inv_sqrt_d = np.float32(1.0 / math.sqrt(HEAD_DIM))
    bf = ml_dtypes.bfloat16
    WqT = np.ascontiguousarray((Wq * inv_sqrt_d).T.astype(bf))
    WkT = np.ascontiguousarray(Wk.T.astype(bf))
    WvT = np.ascontiguousarray(Wv.T.astype(bf))
    WoT = np.ascontiguousarray(Wo.T.astype(bf))
    bq = bq * inv_sqrt_d

    mask = np.asarray(attn_mask)
    with_mask = bool(mask.any())
    maskadd = None
    if with_mask:
        maskadd = np.ascontiguousarray(
            (mask.T.astype(np.float32) * MASK_BIG).astype(bf))

    in_maps = []
    per_batch = {}
    for b in range(B_):
        xbat = x[b]
        norms = np.linalg.norm(xbat, axis=1, keepdims=True)
        xhat = xbat / np.maximum(norms, 1e-12)
        per_batch[b] = (
            np.ascontiguousarray(xbat.T.astype(bf)),
            np.ascontiguousarray(
                (xhat.T * XQ8_SCALE).astype(ml_dtypes.float8_e4m3)),
        )
    for core in range(N_CORES):
        b, g = divmod(core, groups)
        xbT, xq8T = per_batch[b]
        ch = slice(g * HPC * HEAD_DIM, (g + 1) * HPC * HEAD_DIM)
        m = {
            "xb": xbT,
            "xq8": xq8T,
            "wq": np.ascontiguousarray(WqT[:, ch]),
            "wk": np.ascontiguousarray(WkT[:, ch]),
            "wv": np.ascontiguousarray(WvT[:, ch]),
            "wo": np.ascontiguousarray(WoT[ch, :]),
            "bq": np.ascontiguousarray(bq[ch]).reshape(-1, 1),
            "bk": np.ascontiguousarray(bk[ch]).reshape(-1, 1),
            "bv": np.ascontiguousarray(bv[ch]).reshape(-1, 1),
        }
        if with_mask:
            m["maskadd"] = maskadd
        in_maps.append(m)
    return in_maps, with_mask


_NC_CACHE = {}


def _get_nc(with_mask):
    key = with_mask
    if key not in _NC_CACHE:
        _NC_CACHE[key] = build_nc(with_mask=with_mask)
    return _NC_CACHE[key]


LAST_RESULTS = None


def kernel(**inputs):
    global LAST_RESULTS
    in_maps, with_mask = host_prepare(
        inputs["x"], inputs["attn_mask"],
        inputs["Wq"], inputs["bq"], inputs["Wk"], inputs["bk"],
        inputs["Wv"], inputs["bv"], inputs["Wo"], inputs["bo"],
    )
    nc = _get_nc(with_mask)
    res = run_bass_kernel_spmd(nc, in_maps, core_ids=list(range(N_CORES)))
    LAST_RESULTS = res
    bo = np.asarray(inputs["bo"], np.float32)
    out = np.zeros((B, S, HIDDEN), np.float32)
    groups = N_CORES // B
    for core in range(N_CORES):
        b = core // groups
        out[b] += res.results[core]["out"]
    out += bo[None, None, :]
    return out
